# revision 56
# baseline (speedup 1.0000x reference)
# GIN encoder (2x GINConv + BN + global_add_pool) on 8 Trainium2 NeuronCores.
#
# Sharding: nodes and edges are partitioned by destination-node owner
# (12500 nodes/core). Within each core, nodes are permuted into 64-wide
# scatter blocks by a balance heuristic (snake-deal + swap hill-climb on
# per-(block, source-range) edge counts) so the SPMD-shared segment
# lengths (max over cores) sit ~1 above the mean - gather slots run
# within 1% of the per-edge floor. The permutation is applied to the
# gather tables (x_pair host-side, x0p on device), so both layers share
# one slot structure.
#
# Slot layout is range-major: per 25000-row source range (int16 gather
# index limit), all blocks' segments sit back-to-back and dma_gather
# calls are 1024-slot windows (the ucode cap per call) that ignore group
# boundaries, minimizing the 994ns-per-call Pool fixed overhead - the
# Pool engine gather stream is the critical path at ~94% occupancy.
# Gathered bf16 rows are scattered into per-block PSUM windows with
# one-hot matmuls; one-hots are built in a [slots, WB, col] layout whose
# operands all keep a packed 2-byte last dim (DVE 2x mode), ~2x cheaper
# than broadcasting along the window axis.
#
# The MLP runs fully in bf16 (1 PE cycle/row vs 4 for fp32): tanh writes
# bf16 directly, which also feeds the node-major transpose without a
# staging copy. Layer-0 BN is folded into layer-1's first Linear
# (weights scaled by AllReduced inv0 on device, plus a degree-driven
# bias term). Pooling runs in-loop (one-hot matmul per block) and ships
# RAW per-core pool sums plus per-core BN (sum, sumsq) stats; the host
# overlap-adds the windows and applies both layers' BN affine during
# unsharding, so only one AllReduce (+ the x0 AllGather) remains and the
# device tail after the last gather is <1us.

import hashlib
import numpy as np
import ml_dtypes

N_NODES = 100000
N_EDGES = 1000000
D = 64
NUM_GRAPHS = 512
BN_EPS = 1e-5

N_CORES = 8
P = 128
N_LOC = N_NODES // N_CORES          # 12500
NB = (N_LOC + P - 1) // P           # 98 blocks/core
N_PAD = NB * P                      # 12544
RANGE = 25000                       # balanced source ranges (int16-safe)
WB = 64                             # scatter window width (dst nodes)
NB2 = N_LOC // WB + (1 if N_LOC % WB else 0)  # 196 64-wide blocks
GROUP_BLOCKS = 8                    # 128-blocks per gather-call group
NG = (NB + GROUP_BLOCKS - 1) // GROUP_BLOCKS  # 13 groups
ST_BLOCKS = 4                       # blocks per PSUM supertile
NST = (NB + ST_BLOCKS - 1) // ST_BLOCKS       # 25 supertiles
CB = 8                              # pooling one-hot columns per DVE op
CB_OH = 32                          # scatter one-hot columns per DVE op
CALL_CHUNKS = 8                     # chunks per gather call (1024-idx ucode cap)
DMA_RING = 16384                    # SWDGE ring: 1024 descriptors (2048 races on HW)
GBUFS = 30                          # live gather-call tiles (1KB/partition each)

BF16 = ml_dtypes.bfloat16

_cache = {}


def _wrap16(vals):
    # dma_gather index layout: slot i -> [partition i%16, free i//16], x8 copies
    n = vals.shape[0]
    assert n % 16 == 0
    blk = vals.astype(np.int16).reshape(n // 16, 16).T  # [16, n//16]
    return np.tile(blk, (8, 1))  # [128, n//16]


def _balance_positions(deg4, seed=0):
    # deg4: [N_LOC, n_ranges] in-degree by source range for one core's
    # nodes. Assign nodes to 64-wide blocks so per-(block, range) edge
    # counts are near-uniform: the shared (max over cores) segment
    # lengths then sit ~1 above the mean, minimizing gather padding.
    # Snake-deal by total degree, then batched swap hill-climbing on the
    # squared per-(block, range) deviation (reaches ~0.7 mean abs dev).
    rs = np.random.RandomState(seed)
    n_ranges = deg4.shape[1]
    cap = np.full(NB2, WB, dtype=np.int64)
    cap[-1] = N_LOC - WB * (NB2 - 1)
    order = np.argsort(-deg4.sum(1), kind="stable")
    blk = np.empty(N_LOC, dtype=np.int64)
    caps_left = cap.copy()
    fwd = np.arange(NB2)
    i = rounds = 0
    while i < N_LOC:
        cyc = fwd if rounds % 2 == 0 else fwd[::-1]
        for b in cyc:
            if caps_left[b] > 0 and i < N_LOC:
                blk[order[i]] = b
                caps_left[b] -= 1
                i += 1
        rounds += 1
    tgt = deg4.sum(0).astype(np.float64) / NB2
    dev = np.zeros((NB2, n_ranges))
    np.add.at(dev, (blk,), deg4.astype(np.float64))
    dev -= tgt[None, :]
    f = deg4.astype(np.float64)
    for it in range(400):
        u = rs.randint(0, N_LOC, 8192)
        v = rs.randint(0, N_LOC, 8192)
        bu, bv = blk[u], blk[v]
        du = f[u] - f[v]
        old = (dev[bu] ** 2).sum(1) + (dev[bv] ** 2).sum(1)
        new = ((dev[bu] - du) ** 2).sum(1) + ((dev[bv] + du) ** 2).sum(1)
        gain = old - new
        cand = np.where((bu != bv) & (gain > 1e-9))[0]
        if not len(cand):
            continue
        cand = cand[np.argsort(-gain[cand])]
        touched = np.zeros(NB2, dtype=bool)
        for ci in cand:
            a, b2 = bu[ci], bv[ci]
            if touched[a] or touched[b2]:
                continue
            touched[a] = touched[b2] = True
            blk[u[ci]], blk[v[ci]] = b2, a
            dev[a] -= du[ci]
            dev[b2] += du[ci]
    # positions: nodes of block b occupy [64b, 64b + fill_b)
    pos_of = np.empty(N_LOC, dtype=np.int64)
    for b in range(NB2):
        nodes = np.where(blk == b)[0]
        pos_of[nodes] = b * WB + np.arange(len(nodes))
    return pos_of


def _prep_structure(edge_index, batch):
    src = np.asarray(edge_index[0], dtype=np.int64)
    dst = np.asarray(edge_index[1], dtype=np.int64)
    batch = np.asarray(batch, dtype=np.int64)
    n_ranges = (N_NODES + RANGE - 1) // RANGE

    # ---- balanced within-core node permutation ----
    deg4 = np.zeros((N_NODES, n_ranges), dtype=np.int64)
    np.add.at(deg4, (dst, src // RANGE), 1)
    pos_of = np.empty(N_NODES, dtype=np.int64)   # node -> position in core
    for k in range(N_CORES):
        sl = slice(k * N_LOC, (k + 1) * N_LOC)
        pos_of[sl] = _balance_positions(deg4[sl])
    gpos = (np.arange(N_NODES) // N_LOC) * N_LOC + pos_of
    inv_perm = np.empty(N_NODES, dtype=np.int64)
    inv_perm[gpos] = np.arange(N_NODES)          # orig node at each position

    owner = dst // N_LOC
    dst_loc = pos_of[dst]
    block = dst_loc // WB           # 64-wide scatter block
    loc = dst_loc % WB
    rows = gpos[src]                # gather row (x_pair/x0p both permuted)
    rng = rows // RANGE             # == src // RANGE (perm stays in-slab)

    # both layers gather by permuted src id, so they share one structure
    layers = []
    for L in range(1):
        # sort edges by (core, range, block, row)
        order = np.lexsort((rows, block, rng, owner))
        so, sb, sr, srow, sloc = (
            owner[order], block[order], rng[order], rows[order], loc[order])

        # counts per (core, block, range); shared segment length = max
        cnt = np.zeros((N_CORES, NB2, n_ranges), dtype=np.int64)
        np.add.at(cnt, (so, sb, sr), 1)
        seg_len = cnt.max(axis=0)  # [NB2, n_ranges]

        # per-(core,range,block) start offsets in the sorted edge array
        key = (so * n_ranges + sr) * NB2 + sb
        bounds = np.searchsorted(
            key, np.arange(N_CORES * n_ranges * NB2 + 1))

        # range-major slot layout: per range, all groups' segments sit
        # back-to-back and gather calls are 1024-slot windows that ignore
        # group boundaries (only the range tail pads to a 128 multiple)
        seg_off = np.zeros((NB2, n_ranges), dtype=np.int64)
        own_parts = []
        range_span = []
        calls = []      # (r, lo, hi)
        pos = 0
        for r in range(n_ranges):
            base = pos
            for b in range(NB2):
                seg_off[b, r] = pos
                n = int(seg_len[b, r])
                own_parts.append(np.full(n, b, dtype=np.int64))
                pos += n
            used = pos - base
            slots_r = ((used + P - 1) // P) * P
            own_parts.append(np.full(slots_r - used, -1, dtype=np.int64))
            pos = base + slots_r
            range_span.append((base, pos))
            lo = base
            while lo < pos:
                hi = min(lo + CALL_CHUNKS * P, pos)
                calls.append((r, lo, hi))
                lo = hi
        total_slots = pos
        total_chunks = total_slots // P
        seg_own = np.concatenate(own_parts)
        assert seg_own.shape[0] == total_slots

        # first group that consumes each call (pads inherit the previous
        # block's group), for issue scheduling in the device loop
        g_of_slot = np.where(seg_own >= 0, seg_own // (2 * GROUP_BLOCKS), -1)
        for i in range(1, total_slots):
            if g_of_slot[i] < 0:
                g_of_slot[i] = g_of_slot[i - 1]
        call_info = []  # (r, lo, hi, ico, gfirst)
        ico = 0
        for (r, lo, hi) in calls:
            call_info.append((r, lo, hi, ico, int(g_of_slot[lo])))
            ico += (hi - lo) // 16

        # matmul list, block-major within each group so every PSUM window's
        # accumulation group closes before the next one opens in the same
        # bank: per (block, range), one entry per chunk overlapping the
        # block's segment (straddling chunks appear under several blocks,
        # each with its own masked one-hot column)
        call_lo_arr = np.array([lo for (_, lo, hi) in calls])
        mm = []  # (g, b, cid, col, slot0)
        n_mm_b = np.zeros(NB2, dtype=np.int64)
        for g in range(NG):
            blo, bhi = g * 2 * GROUP_BLOCKS, min((g + 1) * 2 * GROUP_BLOCKS, NB2)
            for b in range(blo, bhi):
                for r in range(n_ranges):
                    s, e = int(seg_off[b, r]), int(seg_off[b, r] + seg_len[b, r])
                    if s == e:
                        continue
                    for s0 in range((s // P) * P, e, P):
                        cid = int(np.searchsorted(
                            call_lo_arr, s0, side="right")) - 1
                        _, clo, chi = calls[cid]
                        assert clo <= s0 < chi
                        col = (s0 - clo) // P
                        mm.append((g, b, cid, col, s0))
                        n_mm_b[b] += 1
        assert n_mm_b.min() > 0

        # per-core slot arrays
        idx16_cores, gloc_cores = [], []
        for k in range(N_CORES):
            rows_sl = np.zeros(total_slots, dtype=np.int64)
            gloc_sl = np.full(total_slots, 255, dtype=np.int64)
            for r, (base, end) in enumerate(range_span):
                rows_sl[base:end] = r * RANGE
            for b in range(NB2):
                for r in range(n_ranges):
                    gi = (k * n_ranges + r) * NB2 + b
                    e0, e1 = bounds[gi], bounds[gi + 1]
                    n = e1 - e0
                    s0 = int(seg_off[b, r])
                    cap = int(seg_len[b, r])
                    assert n <= cap
                    rows_sl[s0:s0 + n] = srow[e0:e1]
                    gloc_sl[s0:s0 + n] = sloc[e0:e1]
                    if n < cap:
                        dummy = srow[e1 - 1] if n > 0 else r * RANGE
                        rows_sl[s0 + n:s0 + cap] = dummy
            # per-call int16 local indices
            parts = []
            for (r, lo, hi, _, _) in call_info:
                v = rows_sl[lo:hi] - r * RANGE
                assert v.min() >= 0 and v.max() < RANGE
                parts.append(_wrap16(v))
            idx16_cores.append(np.concatenate(parts, axis=1))
            # gloc columns, one per matmul, masked to the matmul's block
            ga = np.full((len(mm), P), 255, dtype=np.int64)
            for mi, (_, b, _, _, s0) in enumerate(mm):
                sl = slice(s0, s0 + P)
                msk = (seg_own[sl] == b)
                ga[mi][msk] = gloc_sl[sl][msk]
            gloc_cores.append(ga.T.astype(BF16))  # [128, n_mm]

        layers.append(dict(
            calls=call_info, total_chunks=total_chunks,
            total_slots=total_slots, idx16=idx16_cores, gloc=gloc_cores,
            n_rows=N_NODES, n_ranges=n_ranges, mm=mm, n_mm_b=n_mm_b,
        ))
    layers.append(layers[0])

    # pooling: per-core graph windows + per-graph local node counts
    # (slot p of core k holds original node inv_perm[k*N_LOC + p])
    graph_base = []
    ploc_cores = []
    for k in range(N_CORES):
        bs = batch[inv_perm[k * N_LOC:(k + 1) * N_LOC]]
        gb = int(batch[k * N_LOC])
        gl = bs - gb
        assert gl.min() >= 0 and gl.max() < P, "graph window exceeds 128"
        graph_base.append(gb)
        plp = np.full(N_PAD, 255, dtype=np.int64)
        plp[:N_LOC] = gl
        ploc_cores.append(plp.reshape(NB, P).T.astype(BF16))  # [128, NB]

    # deg+1 per local slot, packed [NST, 512] (supertile-major)
    deg = np.bincount(dst, minlength=N_NODES).astype(np.float32)
    degp = []
    for k in range(N_CORES):
        d = np.ones(NST * ST_BLOCKS * P, dtype=np.float32)
        d[:N_LOC] = deg[inv_perm[k * N_LOC:(k + 1) * N_LOC]] + 1.0
        degp.append(d.reshape(NST, ST_BLOCKS * P))

    return dict(layers=layers, graph_base=graph_base, ploc=ploc_cores,
                degp=degp, inv_perm=inv_perm)


def _relax_dma_gather():
    # allow 128B gather elems (HW-verified; the %256 assert is a transpose-
    # mode restriction)
    import inspect, textwrap
    import concourse.bass as bass
    if getattr(bass.BassGpSimd.dma_gather, "_relaxed", False):
        return
    src = inspect.getsource(bass.BassGpSimd.dma_gather)
    src = textwrap.dedent(src)
    src = src.replace("""    assert (
        elem_size_bytes > 0 and elem_size_bytes % 256 == 0
    )  # transpose restriction""", """    assert elem_size_bytes > 0""")
    assert "transpose restriction" not in src
    ns = vars(bass).copy()
    exec(compile("from concourse.bass import *\n" + src, "<dg128>", "exec"), ns)
    f = ns["dma_gather"]
    f._relaxed = True
    bass.BassGpSimd.dma_gather = f


def _build_program(struct, skip_cc=False, max_groups=None, max_layers=2,
                   skip_post=False):
    import concourse.bass as bass
    _relax_dma_gather()
    import concourse.tile as tile
    from concourse import bacc, mybir
    from concourse.masks import make_identity

    FP32 = mybir.dt.float32
    BF = mybir.dt.bfloat16
    I16 = mybir.dt.int16
    AOT = mybir.AluOpType
    ACT = mybir.ActivationFunctionType

    L0, L1 = struct["layers"]
    nc = bacc.Bacc("TRN2", target_bir_lowering=False, debug=False,
                   num_devices=N_CORES, dynamic_dma_scratch_size=DMA_RING)

    # ---- I/O tensors ----
    x_pair_t = nc.dram_tensor("x_pair", [N_NODES, 2 * D], BF, kind="ExternalInput")
    xT_own_t = nc.dram_tensor("xT_own", [D, N_PAD], BF, kind="ExternalInput")
    idx1_t = nc.dram_tensor("idx_l0", [P, L0["idx16"][0].shape[1]], I16,
                            kind="ExternalInput")
    gloc1_t = nc.dram_tensor("gloc_l0", [P, len(L0["mm"])], BF,
                             kind="ExternalInput")
    idx_t = [idx1_t, idx1_t]
    gloc_t = [gloc1_t, gloc1_t]
    ploc_t = nc.dram_tensor("ploc", [P, NB], BF, kind="ExternalInput")
    iota_t = nc.dram_tensor("iotab", [P, CB * P], BF, kind="ExternalInput")
    iota3_t = nc.dram_tensor("iota3", [P, WB * CB_OH], BF, kind="ExternalInput")
    degp_t = nc.dram_tensor("degp", [NST, ST_BLOCKS * P], BF, kind="ExternalInput")
    w1s_t = [nc.dram_tensor(f"w1s_{i}", [2 * D, D], BF, kind="ExternalInput")
             for i in range(2)]
    w2_t = [nc.dram_tensor(f"w2_{i}", [D, D], BF, kind="ExternalInput")
            for i in range(2)]
    b1_t = [nc.dram_tensor(f"b1_{i}", [D, 1], FP32, kind="ExternalInput")
            for i in range(2)]
    b2_t = [nc.dram_tensor(f"b2_{i}", [D, 1], FP32, kind="ExternalInput")
            for i in range(2)]
    gam_t = [nc.dram_tensor(f"gamma_{i}", [D, 1], FP32, kind="ExternalInput")
             for i in range(2)]
    bet_t = [nc.dram_tensor(f"beta_{i}", [D, 1], FP32, kind="ExternalInput")
             for i in range(2)]
    # raw per-core pool sums; BN affine applied on the host during unshard
    out_t = nc.dram_tensor("pool", [P, 2 * D], FP32, kind="ExternalOutput")

    # internal DRAM
    # x0p rows are 256B-pitch (gather stride constraint) but only the
    # first 64 bf16 cols carry data; stored in permuted node order
    x0p_own = nc.dram_tensor("x0p_own", [N_LOC, 2 * D], BF)
    x0p_full = nc.dram_tensor("x0p_full", [N_NODES, 2 * D], BF,
                              addr_space="Local" if skip_cc else "Shared")
    # per-core (sum, sumsq) BN stats: layer 0 is AllReduced on device (L1
    # folds BN0 into its weights); both layers' raw stats go to the host
    bn_in = [nc.dram_tensor(f"bn_in_{i}", [D, 2], FP32) for i in range(2)]
    bn_stat = [nc.dram_tensor(f"bnstat_{i}", [D, 2], FP32,
                              kind="ExternalOutput") for i in range(2)]
    bn_out = [nc.dram_tensor(f"bn_out_{i}", [D, 2], FP32, addr_space="Shared")
              for i in range(2)]

    with tile.TileContext(nc) as tc:
        with tc.tile_pool(name="const", bufs=1) as cpool, \
             tc.tile_pool(name="big", bufs=1) as bigp, \
             tc.tile_pool(name="gbuf", bufs=GBUFS) as gpool, \
             tc.tile_pool(name="work", bufs=3) as wpool, \
             tc.tile_pool(name="oh", bufs=6) as ohpool, \
             tc.tile_pool(name="psA", bufs=2, space="PSUM") as psA, \
             tc.tile_pool(name="psB", bufs=2, space="PSUM") as psB, \
             tc.tile_pool(name="psC", bufs=2, space="PSUM") as psC:

            # ---- constants ----
            iota_b = cpool.tile([P, CB * P], BF)
            nc.scalar.dma_start(iota_b[:], iota_t.ap()[:, :])
            ident = cpool.tile([D, D], BF)
            make_identity(nc, ident[:])
            eps2 = cpool.tile([2 * D, 1], FP32)
            nc.vector.memset(eps2[:], BN_EPS)
            ploc_sb = cpool.tile([P, NB, 1], BF)
            nc.scalar.dma_start(ploc_sb[:, :, 0], ploc_t.ap()[:, :])
            poh_all = cpool.tile([P, NB, P], BF)
            for c0_ in range(0, NB, CB):
                n_ = min(CB, NB - c0_)
                nc.vector.tensor_tensor(
                    out=poh_all[:, c0_:c0_ + n_, :],
                    in0=iota_b[:].rearrange("p (c s) -> p c s", c=CB)[:, :n_, :],
                    in1=ploc_sb[:, c0_:c0_ + n_, :].to_broadcast([P, n_, P]),
                    op=AOT.is_equal)
            w1s_sb, w2_sb, b1_sb, b2_sb = [], [], [], []
            for i in range(2):
                t = cpool.tile([2 * D, D], BF, tag="w1s")
                nc.scalar.dma_start(t[:], w1s_t[i].ap()[:, :]); w1s_sb.append(t)
                t = cpool.tile([D, D], BF, tag="w2")
                nc.scalar.dma_start(t[:], w2_t[i].ap()[:, :]); w2_sb.append(t)
                for lst, tt, tag in ((b1_sb, b1_t, "b1"), (b2_sb, b2_t, "b2")):
                    t = cpool.tile([D, 1], FP32, tag=tag)
                    nc.scalar.dma_start(t[:], tt[i].ap()[:, :]); lst.append(t)

            # persistent activations (bf16: MLP runs fully in bf16)
            hT0 = bigp.tile([D, N_PAD], BF, tag="hT0")
            pool_ps_l = [None, None]  # per-layer PSUM pool accumulators
            # L1 inputs derived from L0 BN (filled between layers)
            w1sc = bigp.tile([2 * D, D], BF, tag="w1sc")
            vecd = bigp.tile([1, D], BF, tag="vecd")
            osb = bigp.tile([P, 2 * D], FP32, tag="osb")

            idx_cols_max = max(
                max((hi - lo) // 16 for (_, lo, hi, _, _) in Ld["calls"])
                for Ld in (L0, L1))

            def bn_coeffs(Li):
                # load AllReduced (sum, sumsq) and produce inv/nbias [D, 1]
                src = (bn_in[Li] if skip_cc else bn_out[Li]).ap()
                bng = wpool.tile([D, 2], FP32, tag="bng")
                nc.sync.dma_start(bng[:], src[:, :])
                gamp = wpool.tile([D, 1], FP32, tag="gamp")
                nc.scalar.dma_start(gamp[:], gam_t[Li].ap()[:, :])
                betp = wpool.tile([D, 1], FP32, tag="betp")
                nc.scalar.dma_start(betp[:], bet_t[Li].ap()[:, :])
                sc = wpool.tile([D, 2], FP32, tag="mu", bufs=2)
                nc.scalar.mul(sc[:], bng[:], 1.0 / N_NODES)
                mu = sc[:, 0:1]
                var = wpool.tile([D, 1], FP32, tag="var")
                nc.vector.tensor_tensor(out=var[:], in0=mu, in1=mu,
                                        op=AOT.mult)
                nc.vector.tensor_tensor(out=var[:], in0=sc[:, 1:2], in1=var[:],
                                        op=AOT.subtract)
                rstd = wpool.tile([D, 1], FP32, tag="rstd")
                nc.scalar.activation(rstd[:], var[:], ACT.Sqrt,
                                     bias=eps2[0:D], scale=1.0)
                nc.vector.reciprocal(rstd[:], rstd[:])
                inv = bigp.tile([D, 1], FP32, tag=f"inv{Li}", name=f"inv{Li}")
                nc.vector.tensor_tensor(out=inv[:], in0=rstd[:], in1=gamp[:],
                                        op=AOT.mult)
                nbias = bigp.tile([D, 1], FP32, tag=f"nb{Li}", name=f"nb{Li}")
                nc.vector.tensor_tensor(out=nbias[:], in0=mu, in1=inv[:],
                                        op=AOT.mult)
                nc.vector.tensor_tensor(out=nbias[:], in0=betp[:],
                                        in1=nbias[:], op=AOT.subtract)
                return inv, nbias

            gloc_sb = cpool.tile([P, len(L0["mm"])], BF)
            nc.scalar.dma_start(gloc_sb[:], gloc_t[0].ap()[:, :])
            iota3_sb = cpool.tile([P, WB, CB_OH], BF)
            nc.scalar.dma_start(
                iota3_sb[:].rearrange("p w c -> p (w c)"), iota3_t.ap()[:, :])

            def layer(Li, Ld):
                calls = Ld["calls"]
                mm = Ld["mm"]
                n_mm_b = Ld["n_mm_b"]

                table = x_pair_t.ap() if Li == 0 else x0p_full.ap()
                n_rows = Ld["n_rows"]

                stats_p = wpool.tile([D, NST, 6], FP32, tag="statsp")
                pool_ps = psC.tile([P, 2 * D], FP32, tag=f"pool{Li}",
                                   name=f"pool{Li}", bufs=1)
                pool_ps_l[Li] = pool_ps
                gci = [0]
                call_tile = {}
                mm_by_g = {}
                for e in mm:
                    mm_by_g.setdefault(e[0], []).append(e)
                seen_b = np.zeros(NB2, dtype=np.int64)

                ngrun = NG if max_groups is None else min(NG, max_groups)
                for g in range(ngrun):
                    blo, bhi = (g * 2 * GROUP_BLOCKS,
                                min((g + 1) * 2 * GROUP_BLOCKS, NB2))
                    # gathers first consumed by this group
                    for cid, (r, lo, hi, ico, gf) in enumerate(calls):
                        if gf != g:
                            continue
                        S = hi - lo
                        it = wpool.tile([P, idx_cols_max], I16, tag="idx",
                                        bufs=12)
                        nc.sync.dma_start(
                            it[:, :S // 16],
                            idx_t[Li].ap()[:, ico:ico + S // 16])
                        gt = gpool.tile([P, CALL_CHUNKS, D], BF, tag="gb")
                        base = r * RANGE
                        nrows_r = min(RANGE, n_rows - base)
                        # single-bf16 rows: gather 128B elems from the
                        # 256B-pitch table
                        nc.gpsimd.dma_gather(
                            gt[:, :S // P, :],
                            table[base:base + nrows_r, 0:D],
                            it[:, :S // 16],
                            S, S, D, elem_step=2 * D,
                        )
                        call_tile[cid] = gt

                    # scatter matmuls for this group
                    chl = mm_by_g.get(g, [])
                    ci0 = gci[0]
                    sts = sorted(set(b // (2 * ST_BLOCKS)
                                     for b in range(blo, bhi)))
                    stp = {st: psA.tile([P, ST_BLOCKS * P], FP32, tag="agg",
                                        name=f"agg{st}")
                           for st in sts}

                    oh_tiles = []
                    ng_ch = len(chl)
                    for cb0 in range(0, ng_ch, CB_OH):
                        n = min(CB_OH, ng_ch - cb0)
                        oh = ohpool.tile([P, WB, CB_OH], BF, tag="oh")
                        # [P, WB, CB] layout: all operands keep a packed
                        # (stride-1, 2-byte) last dim, unlocking the DVE
                        # 2x mode the old broadcast-last layout missed
                        nc.vector.tensor_tensor(
                            out=oh[:, :, :n],
                            in0=iota3_sb[:, :, :n],
                            in1=gloc_sb[:, ci0 + cb0:ci0 + cb0 + n]
                                .rearrange("p (x c) -> p x c", x=1)
                                .to_broadcast([P, WB, n]),
                            op=AOT.is_equal,
                        )
                        oh_tiles.append(oh)

                    for ci, (_, b, cid, col, _) in enumerate(chl):
                        gt = call_tile[cid]
                        oh = oh_tiles[ci // CB_OH]
                        st = b // (2 * ST_BLOCKS)
                        win = (b % (2 * ST_BLOCKS)) * WB
                        first = seen_b[b] == 0
                        last = seen_b[b] == n_mm_b[b] - 1
                        seen_b[b] += 1
                        nc.tensor.matmul(
                            stp[st][0:D, win:win + WB],
                            lhsT=gt[:, col, :],
                            rhs=oh[:, :, ci % CB_OH],
                            start=first, stop=last,
                        )
                    gci[0] += ng_ch

                    if skip_post:
                        continue
                    # supertile post-processing: copy, MLP, h, stats, pool
                    for st in sts:
                        sb0 = st * ST_BLOCKS
                        nwin = min(ST_BLOCKS, NB - sb0) * P
                        c0, c1 = sb0 * P, sb0 * P + nwin
                        agg_sb = wpool.tile([P, ST_BLOCKS * P], BF,
                                            tag="aggsb", bufs=2)
                        nc.scalar.copy(agg_sb[0:D, :nwin],
                                       stp[st][0:D, :nwin])
                        h1p = psB.tile([D, ST_BLOCKS * P], FP32, tag="mlp")
                        # bf16 matmuls: 1 cycle/row (vs 4 for fp32)
                        if Li == 0:
                            nc.tensor.matmul(h1p[:, :nwin],
                                             lhsT=w1s_sb[0][0:D, :],
                                             rhs=agg_sb[0:D, :nwin],
                                             start=True, stop=False)
                            xsl = wpool.tile([D, ST_BLOCKS * P], BF,
                                             tag="xsl", bufs=2)
                            nc.sync.dma_start(xsl[:, :nwin],
                                              xT_own_t.ap()[:, c0:c1])
                            nc.tensor.matmul(h1p[:, :nwin],
                                             lhsT=w1s_sb[0][0:D, :],
                                             rhs=xsl[:, :nwin],
                                             start=False, stop=True)
                        else:
                            nc.tensor.matmul(h1p[:, :nwin],
                                             lhsT=w1sc[0:D, :],
                                             rhs=agg_sb[0:D, :nwin],
                                             start=True, stop=False)
                            nc.tensor.matmul(h1p[:, :nwin],
                                             lhsT=w1sc[0:D, :],
                                             rhs=hT0[:, c0:c1],
                                             start=False, stop=False)
                            dsl = wpool.tile([1, ST_BLOCKS * P], BF,
                                             tag="dsl", bufs=2)
                            nc.sync.dma_start(dsl[:, :nwin],
                                              degp_t.ap()[st:st + 1, :nwin])
                            nc.tensor.matmul(h1p[:, :nwin],
                                             lhsT=vecd[:],
                                             rhs=dsl[:, :nwin],
                                             start=False, stop=True)
                        t1 = wpool.tile([D, ST_BLOCKS * P], BF, tag="t1",
                                        bufs=2)
                        nc.scalar.activation(t1[:, :nwin], h1p[:, :nwin],
                                             ACT.Tanh, bias=b1_sb[Li][:],
                                             scale=1.0)
                        h2p = psB.tile([D, ST_BLOCKS * P], FP32, tag="mlp")
                        nc.tensor.matmul(h2p[:, :nwin],
                                         lhsT=w2_sb[Li][:],
                                         rhs=t1[:, :nwin],
                                         start=True, stop=True)
                        if Li == 0:
                            ht_t = None
                            hts = hT0[:, c0:c1]
                        else:
                            ht_t = wpool.tile([D, ST_BLOCKS * P], BF,
                                              tag="ht1", bufs=2)
                            hts = ht_t[:, :nwin]
                        nc.scalar.activation(hts, h2p[:, :nwin],
                                             ACT.Tanh, bias=b2_sb[Li][:],
                                             scale=1.0)
                        # stats partials (exclude padded tail nodes)
                        r1 = min(c1, N_LOC)
                        if c0 < N_LOC:
                            hstat = (hT0[:, c0:r1] if Li == 0
                                     else ht_t[:, :r1 - c0])
                            nc.vector.bn_stats(out=stats_p[:, st, :],
                                               in_=hstat)
                        # bf16 transpose to node-major (PSUM); tanh already
                        # wrote bf16, so no staging copy is needed
                        tp = psC.tile([P, ST_BLOCKS, D], BF, tag="tp", bufs=1)
                        nbl = nwin // P
                        for j in range(nbl):
                            src = (hT0[:, c0 + j * P:c0 + (j + 1) * P]
                                   if Li == 0 else ht_t[:, j * P:(j + 1) * P])
                            nc.tensor.transpose(tp[:, j, :], src, ident[:])
                        xp = wpool.tile([P, ST_BLOCKS, D], BF,
                                        tag="xp", bufs=2)
                        nc.scalar.copy(xp[:, :nbl, :], tp[:, :nbl, :])
                        if Li == 0:
                            # single-bf16 writeback, clipped to N_LOC
                            nfull = max(0, min(c1, N_LOC) - c0) // P
                            if nfull:
                                nc.sync.dma_start(
                                    x0p_own.ap()[c0:c0 + nfull * P, 0:D]
                                    .rearrange("(j p) f -> p j f", p=P),
                                    xp[:, :nfull, :])
                            rem = min(c1, N_LOC) - (c0 + nfull * P)
                            if rem > 0:
                                nc.sync.dma_start(
                                    x0p_own.ap()[c0 + nfull * P:
                                                 c0 + nfull * P + rem, 0:D],
                                    xp[0:rem, nfull, :])
                        # pooling: one-hot matmul per block,
                        # accumulated in PSUM across the whole layer
                        for j in range(nbl):
                            b = sb0 + j
                            nc.tensor.matmul(
                                pool_ps[:, 0:D], lhsT=poh_all[:, b, :],
                                rhs=xp[:, j, :],
                                start=(b == 0), stop=(b == NB - 1))

                if skip_post:
                    return
                # ---- BN stats reduce + AllReduce ----
                mv = wpool.tile([D, 2], FP32, tag="mv")
                nc.vector.bn_aggr(out=mv[:], in_=stats_p[:])
                bpack = wpool.tile([D, 2], FP32, tag="bpack")
                nc.scalar.mul(bpack[:, 0:1], mv[:, 0:1], float(N_LOC))
                msq = wpool.tile([D, 1], FP32, tag="msq")
                nc.vector.tensor_tensor(out=msq[:], in0=mv[:, 0:1],
                                        in1=mv[:, 0:1], op=AOT.mult)
                nc.vector.tensor_tensor(out=msq[:], in0=mv[:, 1:2],
                                        in1=msq[:], op=AOT.add)
                nc.scalar.mul(bpack[:, 1:2], msq[:], float(N_LOC))
                nc.sync.dma_start(bn_stat[Li].ap()[:, :], bpack[:])
                if Li == 0:
                    nc.sync.dma_start(bn_in[0].ap()[:, :], bpack[:])
                if Li == 0 and not skip_cc:
                    # only layer 0's BN stats are needed on device (folded
                    # into L1's weights); L1's BN affine runs on the host
                    nc.gpsimd.collective_compute(
                        "AllReduce", AOT.add,
                        replica_groups=[list(range(N_CORES))],
                        ins=[bn_in[Li].ap().opt()],
                        outs=[bn_out[Li].ap().opt()],
                    )
                if Li == 0 and not skip_cc:
                    nc.gpsimd.collective_compute(
                        "AllGather", AOT.bypass,
                        replica_groups=[list(range(N_CORES))],
                        ins=[x0p_own.ap().opt()],
                        outs=[x0p_full.ap().opt()],
                    )
                # raw pool sums out (host applies the BN affine)
                nc.scalar.copy(osb[:, Li * D:(Li + 1) * D], pool_ps[:, 0:D])
                if Li == 0 and max_layers > 1:
                    inv, nbias = bn_coeffs(0)
                    # scale L1's stacked W1 by inv0; degree-bias row vector
                    nc.vector.tensor_scalar(
                        out=w1sc[0:D, :], in0=w1s_sb[1][0:D, :],
                        scalar1=inv[:], scalar2=None,
                        op0=AOT.mult)
                    nbias_bf = wpool.tile([D, 1], BF, tag="nbbf")
                    nc.vector.tensor_copy(nbias_bf[:], nbias[:])
                    vp = psC.tile([P, P], FP32, tag="misc", bufs=1)
                    nc.tensor.matmul(vp[0:1, 0:D], lhsT=nbias_bf[:],
                                     rhs=w1s_sb[1][0:D, :],
                                     start=True, stop=True)
                    nc.scalar.copy(vecd[:], vp[0:1, 0:D])

            layer(0, L0)
            if max_layers > 1:
                layer(1, L1)

            if not skip_post:
                nc.sync.dma_start(out_t.ap()[:, :], osb[:])

    nc.compile()
    return nc


def kernel(**inputs):
    from concourse.bass_utils import run_bass_kernel_spmd

    edge_index = np.asarray(inputs["edge_index"])
    batch = np.asarray(inputs["batch"])
    key = hashlib.sha1(
        edge_index.tobytes() + batch.tobytes()).hexdigest()
    if key not in _cache:
        struct = _prep_structure(edge_index, batch)
        nc = _build_program(struct)
        _cache[key] = (struct, nc)
    struct, nc = _cache[key]

    x = np.asarray(inputs["x"], dtype=np.float32)
    inv_perm = struct["inv_perm"]
    xp_all = x[inv_perm]                       # permuted node order
    x_pair = np.zeros((N_NODES, 2 * D), dtype=BF16)
    x_pair[:, 0:D] = xp_all.astype(BF16)
    in_maps = []
    for k in range(N_CORES):
        xT_own = np.zeros((D, N_PAD), dtype=BF16)
        xT_own[:, :N_LOC] = xp_all[k * N_LOC:(k + 1) * N_LOC].T.astype(BF16)
        m = dict(
            x_pair=x_pair,
            xT_own=xT_own,
            iotab=np.ascontiguousarray(
                np.tile(np.arange(P, dtype=np.float32), (P, CB))
                .reshape(P, CB * P).astype(BF16)),
            iota3=np.ascontiguousarray(np.tile(
                np.repeat(np.arange(WB, dtype=np.float32), CB_OH),
                (P, 1)).astype(BF16)),
            ploc=np.ascontiguousarray(struct["ploc"][k]),
            degp=np.ascontiguousarray(struct["degp"][k].astype(BF16)),
        )
        Ld = struct["layers"][0]
        m["idx_l0"] = np.ascontiguousarray(Ld["idx16"][k])
        m["gloc_l0"] = np.ascontiguousarray(Ld["gloc"][k])
        for i in range(2):
            W1 = np.asarray(inputs[f"W1_{i}"], dtype=np.float32)
            m[f"w1s_{i}"] = np.concatenate([W1, W1], axis=0).astype(BF16)
            m[f"w2_{i}"] = np.asarray(inputs[f"W2_{i}"], dtype=np.float32).astype(BF16)
            m[f"b1_{i}"] = np.asarray(inputs[f"b1_{i}"], dtype=np.float32).reshape(D, 1)
            m[f"b2_{i}"] = np.asarray(inputs[f"b2_{i}"], dtype=np.float32).reshape(D, 1)
            m[f"gamma_{i}"] = np.asarray(inputs[f"gamma_{i}"], dtype=np.float32).reshape(D, 1)
            m[f"beta_{i}"] = np.asarray(inputs[f"beta_{i}"], dtype=np.float32).reshape(D, 1)
        in_maps.append(m)

    res = run_bass_kernel_spmd(nc, in_maps, core_ids=list(range(N_CORES)))
    kernel.last_results = res

    # unshard: overlap-add raw per-core pool sums, then apply the BN
    # affine (device ships raw sums + per-core (sum, sumsq) stats)
    praw = np.zeros((NUM_GRAPHS, 2 * D), dtype=np.float64)
    for k in range(N_CORES):
        gb = struct["graph_base"][k]
        n = min(P, NUM_GRAPHS - gb)
        praw[gb:gb + n] += res.results[k]["pool"][:n]
    gcnt = np.bincount(batch.astype(np.int64),
                       minlength=NUM_GRAPHS).astype(np.float64)
    out = np.zeros((NUM_GRAPHS, 2 * D), dtype=np.float32)
    for i in range(2):
        S = np.zeros((D, 2), dtype=np.float64)
        for k in range(N_CORES):
            S += res.results[k][f"bnstat_{i}"]
        mu = S[:, 0] / N_NODES
        var = S[:, 1] / N_NODES - mu * mu
        gamma = np.asarray(inputs[f"gamma_{i}"], dtype=np.float64)
        beta = np.asarray(inputs[f"beta_{i}"], dtype=np.float64)
        inv = gamma / np.sqrt(var + BN_EPS)
        nbias = beta - mu * inv
        out[:, i * D:(i + 1) * D] = (
            praw[:, i * D:(i + 1) * D] * inv[None, :]
            + gcnt[:, None] * nbias[None, :]).astype(np.float32)
    return out



# revision 64
# speedup vs baseline: 1.0047x; 1.0047x over previous
# GIN encoder (2x GINConv + BN + global_add_pool) on 8 Trainium2 NeuronCores.
#
# Sharding: nodes and edges are partitioned by destination-node owner
# (12500 nodes/core). Within each core, nodes are permuted into 64-wide
# scatter blocks by a balance heuristic (snake-deal + swap hill-climb on
# per-(block, source-range) edge counts) so the SPMD-shared segment
# lengths (max over cores) sit ~1 above the mean - gather slots run
# within 1% of the per-edge floor. The permutation is applied to the
# gather tables (x_pair host-side, x0p on device), so both layers share
# one slot structure.
#
# Slot layout is range-major: per 25000-row source range (int16 gather
# index limit), all blocks' segments sit back-to-back and dma_gather
# calls are 1024-slot windows (the ucode cap per call) that ignore group
# boundaries, minimizing the 994ns-per-call Pool fixed overhead - the
# Pool engine gather stream is the critical path at ~94% occupancy.
# Gathered bf16 rows are scattered into per-block PSUM windows with
# one-hot matmuls; one-hots are built in a [slots, WB, col] layout whose
# operands all keep a packed 2-byte last dim (DVE 2x mode), ~2x cheaper
# than broadcasting along the window axis.
#
# The MLP runs fully in bf16 (1 PE cycle/row vs 4 for fp32): tanh writes
# bf16 directly, which also feeds the node-major transpose without a
# staging copy. Layer-0 BN is folded into layer-1's first Linear
# (weights scaled by AllReduced inv0 on device, plus a degree-driven
# bias term). Pooling runs in-loop (one-hot matmul per block) and ships
# RAW per-core pool sums plus per-core BN (sum, sumsq) stats; the host
# overlap-adds the windows and applies both layers' BN affine during
# unsharding, so only one AllReduce (+ the x0 AllGather) remains and the
# device tail after the last gather is <1us.

import hashlib
import numpy as np
import ml_dtypes

N_NODES = 100000
N_EDGES = 1000000
D = 64
NUM_GRAPHS = 512
BN_EPS = 1e-5

N_CORES = 8
P = 128
N_LOC = N_NODES // N_CORES          # 12500
NB = (N_LOC + P - 1) // P           # 98 blocks/core
N_PAD = NB * P                      # 12544
RANGE = 25000                       # balanced source ranges (int16-safe)
WB = 64                             # scatter window width (dst nodes)
NB2 = N_LOC // WB + (1 if N_LOC % WB else 0)  # 196 64-wide blocks
GROUP_BLOCKS = 4                    # 128-blocks per group (1 supertile)
NG = (NB + GROUP_BLOCKS - 1) // GROUP_BLOCKS  # 13 groups
ST_BLOCKS = 4                       # blocks per PSUM supertile
NST = (NB + ST_BLOCKS - 1) // ST_BLOCKS       # 25 supertiles
CB = 8                              # pooling one-hot columns per DVE op
CB_OH = 32                          # scatter one-hot columns per DVE op
CALL_CHUNKS = 8                     # chunks per gather call (1024-idx ucode cap)
DMA_RING = 16384                    # SWDGE ring: 1024 descriptors (2048 races on HW)
GBUFS = 30                          # live gather-call tiles (1KB/partition each)

BF16 = ml_dtypes.bfloat16

_cache = {}


def _wrap16(vals):
    # dma_gather index layout: slot i -> [partition i%16, free i//16], x8 copies
    n = vals.shape[0]
    assert n % 16 == 0
    blk = vals.astype(np.int16).reshape(n // 16, 16).T  # [16, n//16]
    return np.tile(blk, (8, 1))  # [128, n//16]


def _balance_positions(deg4, seed=0):
    # deg4: [N_LOC, n_ranges] in-degree by source range for one core's
    # nodes. Assign nodes to 64-wide blocks so per-(block, range) edge
    # counts are near-uniform: the shared (max over cores) segment
    # lengths then sit ~1 above the mean, minimizing gather padding.
    # Snake-deal by total degree, then batched swap hill-climbing on the
    # squared per-(block, range) deviation (reaches ~0.7 mean abs dev).
    rs = np.random.RandomState(seed)
    n_ranges = deg4.shape[1]
    cap = np.full(NB2, WB, dtype=np.int64)
    cap[-1] = N_LOC - WB * (NB2 - 1)
    order = np.argsort(-deg4.sum(1), kind="stable")
    blk = np.empty(N_LOC, dtype=np.int64)
    caps_left = cap.copy()
    fwd = np.arange(NB2)
    i = rounds = 0
    while i < N_LOC:
        cyc = fwd if rounds % 2 == 0 else fwd[::-1]
        for b in cyc:
            if caps_left[b] > 0 and i < N_LOC:
                blk[order[i]] = b
                caps_left[b] -= 1
                i += 1
        rounds += 1
    tgt = deg4.sum(0).astype(np.float64) / NB2
    dev = np.zeros((NB2, n_ranges))
    np.add.at(dev, (blk,), deg4.astype(np.float64))
    dev -= tgt[None, :]
    f = deg4.astype(np.float64)
    for it in range(400):
        u = rs.randint(0, N_LOC, 8192)
        v = rs.randint(0, N_LOC, 8192)
        bu, bv = blk[u], blk[v]
        du = f[u] - f[v]
        old = (dev[bu] ** 2).sum(1) + (dev[bv] ** 2).sum(1)
        new = ((dev[bu] - du) ** 2).sum(1) + ((dev[bv] + du) ** 2).sum(1)
        gain = old - new
        cand = np.where((bu != bv) & (gain > 1e-9))[0]
        if not len(cand):
            continue
        cand = cand[np.argsort(-gain[cand])]
        touched = np.zeros(NB2, dtype=bool)
        for ci in cand:
            a, b2 = bu[ci], bv[ci]
            if touched[a] or touched[b2]:
                continue
            touched[a] = touched[b2] = True
            blk[u[ci]], blk[v[ci]] = b2, a
            dev[a] -= du[ci]
            dev[b2] += du[ci]
    # positions: nodes of block b occupy [64b, 64b + fill_b)
    pos_of = np.empty(N_LOC, dtype=np.int64)
    for b in range(NB2):
        nodes = np.where(blk == b)[0]
        pos_of[nodes] = b * WB + np.arange(len(nodes))
    return pos_of


def _prep_structure(edge_index, batch):
    src = np.asarray(edge_index[0], dtype=np.int64)
    dst = np.asarray(edge_index[1], dtype=np.int64)
    batch = np.asarray(batch, dtype=np.int64)
    n_ranges = (N_NODES + RANGE - 1) // RANGE

    # ---- balanced within-core node permutation ----
    deg4 = np.zeros((N_NODES, n_ranges), dtype=np.int64)
    np.add.at(deg4, (dst, src // RANGE), 1)
    pos_of = np.empty(N_NODES, dtype=np.int64)   # node -> position in core
    for k in range(N_CORES):
        sl = slice(k * N_LOC, (k + 1) * N_LOC)
        pos_of[sl] = _balance_positions(deg4[sl])
    gpos = (np.arange(N_NODES) // N_LOC) * N_LOC + pos_of
    inv_perm = np.empty(N_NODES, dtype=np.int64)
    inv_perm[gpos] = np.arange(N_NODES)          # orig node at each position

    owner = dst // N_LOC
    dst_loc = pos_of[dst]
    block = dst_loc // WB           # 64-wide scatter block
    loc = dst_loc % WB
    rows = gpos[src]                # gather row (x_pair/x0p both permuted)
    rng = rows // RANGE             # == src // RANGE (perm stays in-slab)

    # both layers gather by permuted src id, so they share one structure
    layers = []
    for L in range(1):
        # sort edges by (core, range, block, row)
        order = np.lexsort((rows, block, rng, owner))
        so, sb, sr, srow, sloc = (
            owner[order], block[order], rng[order], rows[order], loc[order])

        # counts per (core, block, range); shared segment length = max
        cnt = np.zeros((N_CORES, NB2, n_ranges), dtype=np.int64)
        np.add.at(cnt, (so, sb, sr), 1)
        seg_len = cnt.max(axis=0)  # [NB2, n_ranges]

        # per-(core,range,block) start offsets in the sorted edge array
        key = (so * n_ranges + sr) * NB2 + sb
        bounds = np.searchsorted(
            key, np.arange(N_CORES * n_ranges * NB2 + 1))

        # range-major slot layout: per range, all groups' segments sit
        # back-to-back and gather calls are 1024-slot windows that ignore
        # group boundaries (only the range tail pads to a 128 multiple)
        seg_off = np.zeros((NB2, n_ranges), dtype=np.int64)
        own_parts = []
        range_span = []
        calls = []      # (r, lo, hi)
        pos = 0
        for r in range(n_ranges):
            base = pos
            for b in range(NB2):
                seg_off[b, r] = pos
                n = int(seg_len[b, r])
                own_parts.append(np.full(n, b, dtype=np.int64))
                pos += n
            used = pos - base
            slots_r = ((used + P - 1) // P) * P
            own_parts.append(np.full(slots_r - used, -1, dtype=np.int64))
            pos = base + slots_r
            range_span.append((base, pos))
            lo = base
            while lo < pos:
                hi = min(lo + CALL_CHUNKS * P, pos)
                calls.append((r, lo, hi))
                lo = hi
        total_slots = pos
        total_chunks = total_slots // P
        seg_own = np.concatenate(own_parts)
        assert seg_own.shape[0] == total_slots

        # first group that consumes each call (pads inherit the previous
        # block's group), for issue scheduling in the device loop
        g_of_slot = np.where(seg_own >= 0, seg_own // (2 * GROUP_BLOCKS), -1)
        for i in range(1, total_slots):
            if g_of_slot[i] < 0:
                g_of_slot[i] = g_of_slot[i - 1]
        call_info = []  # (r, lo, hi, ico, gfirst)
        ico = 0
        for (r, lo, hi) in calls:
            call_info.append((r, lo, hi, ico, int(g_of_slot[lo])))
            ico += (hi - lo) // 16

        # matmul list, block-major within each group so every PSUM window's
        # accumulation group closes before the next one opens in the same
        # bank: per (block, range), one entry per chunk overlapping the
        # block's segment (straddling chunks appear under several blocks,
        # each with its own masked one-hot column)
        call_lo_arr = np.array([lo for (_, lo, hi) in calls])
        mm = []  # (g, b, cid, col, slot0)
        n_mm_b = np.zeros(NB2, dtype=np.int64)
        for g in range(NG):
            blo, bhi = g * 2 * GROUP_BLOCKS, min((g + 1) * 2 * GROUP_BLOCKS, NB2)
            for b in range(blo, bhi):
                for r in range(n_ranges):
                    s, e = int(seg_off[b, r]), int(seg_off[b, r] + seg_len[b, r])
                    if s == e:
                        continue
                    for s0 in range((s // P) * P, e, P):
                        cid = int(np.searchsorted(
                            call_lo_arr, s0, side="right")) - 1
                        _, clo, chi = calls[cid]
                        assert clo <= s0 < chi
                        col = (s0 - clo) // P
                        mm.append((g, b, cid, col, s0))
                        n_mm_b[b] += 1
        assert n_mm_b.min() > 0

        # per-core slot arrays
        idx16_cores, gloc_cores = [], []
        for k in range(N_CORES):
            rows_sl = np.zeros(total_slots, dtype=np.int64)
            gloc_sl = np.full(total_slots, 255, dtype=np.int64)
            for r, (base, end) in enumerate(range_span):
                rows_sl[base:end] = r * RANGE
            for b in range(NB2):
                for r in range(n_ranges):
                    gi = (k * n_ranges + r) * NB2 + b
                    e0, e1 = bounds[gi], bounds[gi + 1]
                    n = e1 - e0
                    s0 = int(seg_off[b, r])
                    cap = int(seg_len[b, r])
                    assert n <= cap
                    rows_sl[s0:s0 + n] = srow[e0:e1]
                    gloc_sl[s0:s0 + n] = sloc[e0:e1]
                    if n < cap:
                        dummy = srow[e1 - 1] if n > 0 else r * RANGE
                        rows_sl[s0 + n:s0 + cap] = dummy
            # per-call int16 local indices
            parts = []
            for (r, lo, hi, _, _) in call_info:
                v = rows_sl[lo:hi] - r * RANGE
                assert v.min() >= 0 and v.max() < RANGE
                parts.append(_wrap16(v))
            idx16_cores.append(np.concatenate(parts, axis=1))
            # gloc columns, one per matmul, masked to the matmul's block
            ga = np.full((len(mm), P), 255, dtype=np.int64)
            for mi, (_, b, _, _, s0) in enumerate(mm):
                sl = slice(s0, s0 + P)
                msk = (seg_own[sl] == b)
                ga[mi][msk] = gloc_sl[sl][msk]
            gloc_cores.append(ga.T.astype(BF16))  # [128, n_mm]

        layers.append(dict(
            calls=call_info, total_chunks=total_chunks,
            total_slots=total_slots, idx16=idx16_cores, gloc=gloc_cores,
            n_rows=N_NODES, n_ranges=n_ranges, mm=mm, n_mm_b=n_mm_b,
        ))
    layers.append(layers[0])

    # pooling: per-core graph windows + per-graph local node counts
    # (slot p of core k holds original node inv_perm[k*N_LOC + p])
    graph_base = []
    ploc_cores = []
    for k in range(N_CORES):
        bs = batch[inv_perm[k * N_LOC:(k + 1) * N_LOC]]
        gb = int(batch[k * N_LOC])
        gl = bs - gb
        assert gl.min() >= 0 and gl.max() < P, "graph window exceeds 128"
        graph_base.append(gb)
        plp = np.full(N_PAD, 255, dtype=np.int64)
        plp[:N_LOC] = gl
        ploc_cores.append(plp.reshape(NB, P).T.astype(BF16))  # [128, NB]

    # deg+1 per local slot, packed [NST, 512] (supertile-major)
    deg = np.bincount(dst, minlength=N_NODES).astype(np.float32)
    degp = []
    for k in range(N_CORES):
        d = np.ones(NST * ST_BLOCKS * P, dtype=np.float32)
        d[:N_LOC] = deg[inv_perm[k * N_LOC:(k + 1) * N_LOC]] + 1.0
        degp.append(d.reshape(NST, ST_BLOCKS * P))

    return dict(layers=layers, graph_base=graph_base, ploc=ploc_cores,
                degp=degp, inv_perm=inv_perm)


def _relax_dma_gather():
    # allow 128B gather elems (HW-verified; the %256 assert is a transpose-
    # mode restriction)
    import inspect, textwrap
    import concourse.bass as bass
    if getattr(bass.BassGpSimd.dma_gather, "_relaxed", False):
        return
    src = inspect.getsource(bass.BassGpSimd.dma_gather)
    src = textwrap.dedent(src)
    src = src.replace("""    assert (
        elem_size_bytes > 0 and elem_size_bytes % 256 == 0
    )  # transpose restriction""", """    assert elem_size_bytes > 0""")
    assert "transpose restriction" not in src
    ns = vars(bass).copy()
    exec(compile("from concourse.bass import *\n" + src, "<dg128>", "exec"), ns)
    f = ns["dma_gather"]
    f._relaxed = True
    bass.BassGpSimd.dma_gather = f


def _build_program(struct, skip_cc=False, max_groups=None, max_layers=2,
                   skip_post=False):
    import concourse.bass as bass
    _relax_dma_gather()
    import concourse.tile as tile
    from concourse import bacc, mybir
    from concourse.masks import make_identity

    FP32 = mybir.dt.float32
    BF = mybir.dt.bfloat16
    I16 = mybir.dt.int16
    AOT = mybir.AluOpType
    ACT = mybir.ActivationFunctionType

    L0, L1 = struct["layers"]
    nc = bacc.Bacc("TRN2", target_bir_lowering=False, debug=False,
                   num_devices=N_CORES, dynamic_dma_scratch_size=DMA_RING)

    # ---- I/O tensors ----
    x_pair_t = nc.dram_tensor("x_pair", [N_NODES, 2 * D], BF, kind="ExternalInput")
    xT_own_t = nc.dram_tensor("xT_own", [D, N_PAD], BF, kind="ExternalInput")
    idx1_t = nc.dram_tensor("idx_l0", [P, L0["idx16"][0].shape[1]], I16,
                            kind="ExternalInput")
    gloc1_t = nc.dram_tensor("gloc_l0", [P, len(L0["mm"])], BF,
                             kind="ExternalInput")
    idx_t = [idx1_t, idx1_t]
    gloc_t = [gloc1_t, gloc1_t]
    ploc_t = nc.dram_tensor("ploc", [P, NB], BF, kind="ExternalInput")
    iota_t = nc.dram_tensor("iotab", [P, CB * P], BF, kind="ExternalInput")
    iota3_t = nc.dram_tensor("iota3", [P, WB * CB_OH], BF, kind="ExternalInput")
    degp_t = nc.dram_tensor("degp", [NST, ST_BLOCKS * P], BF, kind="ExternalInput")
    w1s_t = [nc.dram_tensor(f"w1s_{i}", [2 * D, D], BF, kind="ExternalInput")
             for i in range(2)]
    w2_t = [nc.dram_tensor(f"w2_{i}", [D, D], BF, kind="ExternalInput")
            for i in range(2)]
    b1_t = [nc.dram_tensor(f"b1_{i}", [D, 1], FP32, kind="ExternalInput")
            for i in range(2)]
    b2_t = [nc.dram_tensor(f"b2_{i}", [D, 1], FP32, kind="ExternalInput")
            for i in range(2)]
    gam_t = [nc.dram_tensor(f"gamma_{i}", [D, 1], FP32, kind="ExternalInput")
             for i in range(2)]
    bet_t = [nc.dram_tensor(f"beta_{i}", [D, 1], FP32, kind="ExternalInput")
             for i in range(2)]
    # raw per-core pool sums; BN affine applied on the host during unshard
    out_t = nc.dram_tensor("pool", [P, 2 * D], FP32, kind="ExternalOutput")

    # internal DRAM
    # x0p rows are 256B-pitch (gather stride constraint) but only the
    # first 64 bf16 cols carry data; stored in permuted node order
    x0p_own = nc.dram_tensor("x0p_own", [N_LOC, 2 * D], BF)
    x0p_full = nc.dram_tensor("x0p_full", [N_NODES, 2 * D], BF,
                              addr_space="Local" if skip_cc else "Shared")
    # per-core (sum, sumsq) BN stats: layer 0 is AllReduced on device (L1
    # folds BN0 into its weights); both layers' raw stats go to the host
    bn_in = [nc.dram_tensor(f"bn_in_{i}", [D, 2], FP32) for i in range(2)]
    bn_stat = [nc.dram_tensor(f"bnstat_{i}", [D, 2], FP32,
                              kind="ExternalOutput") for i in range(2)]
    bn_out = [nc.dram_tensor(f"bn_out_{i}", [D, 2], FP32, addr_space="Shared")
              for i in range(2)]

    with tile.TileContext(nc) as tc:
        with tc.tile_pool(name="const", bufs=1) as cpool, \
             tc.tile_pool(name="big", bufs=1) as bigp, \
             tc.tile_pool(name="gbuf", bufs=GBUFS) as gpool, \
             tc.tile_pool(name="work", bufs=3) as wpool, \
             tc.tile_pool(name="oh", bufs=6) as ohpool, \
             tc.tile_pool(name="psA", bufs=2, space="PSUM") as psA, \
             tc.tile_pool(name="psB", bufs=2, space="PSUM") as psB, \
             tc.tile_pool(name="psC", bufs=2, space="PSUM") as psC:

            # ---- constants ----
            iota_b = cpool.tile([P, CB * P], BF)
            nc.scalar.dma_start(iota_b[:], iota_t.ap()[:, :])
            ident = cpool.tile([D, D], BF)
            make_identity(nc, ident[:])
            eps2 = cpool.tile([2 * D, 1], FP32)
            nc.vector.memset(eps2[:], BN_EPS)
            ploc_sb = cpool.tile([P, NB, 1], BF)
            nc.scalar.dma_start(ploc_sb[:, :, 0], ploc_t.ap()[:, :])
            poh_all = cpool.tile([P, NB, P], BF)
            for c0_ in range(0, NB, CB):
                n_ = min(CB, NB - c0_)
                nc.vector.tensor_tensor(
                    out=poh_all[:, c0_:c0_ + n_, :],
                    in0=iota_b[:].rearrange("p (c s) -> p c s", c=CB)[:, :n_, :],
                    in1=ploc_sb[:, c0_:c0_ + n_, :].to_broadcast([P, n_, P]),
                    op=AOT.is_equal)
            w1s_sb, w2_sb, b1_sb, b2_sb = [], [], [], []
            for i in range(2):
                t = cpool.tile([2 * D, D], BF, tag="w1s")
                nc.scalar.dma_start(t[:], w1s_t[i].ap()[:, :]); w1s_sb.append(t)
                t = cpool.tile([D, D], BF, tag="w2")
                nc.scalar.dma_start(t[:], w2_t[i].ap()[:, :]); w2_sb.append(t)
                for lst, tt, tag in ((b1_sb, b1_t, "b1"), (b2_sb, b2_t, "b2")):
                    t = cpool.tile([D, 1], FP32, tag=tag)
                    nc.scalar.dma_start(t[:], tt[i].ap()[:, :]); lst.append(t)

            # persistent activations (bf16: MLP runs fully in bf16)
            hT0 = bigp.tile([D, N_PAD], BF, tag="hT0")
            pool_ps_l = [None, None]  # per-layer PSUM pool accumulators
            # L1 inputs derived from L0 BN (filled between layers)
            w1sc = bigp.tile([2 * D, D], BF, tag="w1sc")
            vecd = bigp.tile([1, D], BF, tag="vecd")
            osb = bigp.tile([P, 2 * D], FP32, tag="osb")

            idx_cols_max = max(
                max((hi - lo) // 16 for (_, lo, hi, _, _) in Ld["calls"])
                for Ld in (L0, L1))

            def bn_coeffs(Li):
                # load AllReduced (sum, sumsq) and produce inv/nbias [D, 1]
                src = (bn_in[Li] if skip_cc else bn_out[Li]).ap()
                bng = wpool.tile([D, 2], FP32, tag="bng")
                nc.sync.dma_start(bng[:], src[:, :])
                gamp = wpool.tile([D, 1], FP32, tag="gamp")
                nc.scalar.dma_start(gamp[:], gam_t[Li].ap()[:, :])
                betp = wpool.tile([D, 1], FP32, tag="betp")
                nc.scalar.dma_start(betp[:], bet_t[Li].ap()[:, :])
                sc = wpool.tile([D, 2], FP32, tag="mu", bufs=2)
                nc.scalar.mul(sc[:], bng[:], 1.0 / N_NODES)
                mu = sc[:, 0:1]
                var = wpool.tile([D, 1], FP32, tag="var")
                nc.vector.tensor_tensor(out=var[:], in0=mu, in1=mu,
                                        op=AOT.mult)
                nc.vector.tensor_tensor(out=var[:], in0=sc[:, 1:2], in1=var[:],
                                        op=AOT.subtract)
                rstd = wpool.tile([D, 1], FP32, tag="rstd")
                nc.scalar.activation(rstd[:], var[:], ACT.Sqrt,
                                     bias=eps2[0:D], scale=1.0)
                nc.vector.reciprocal(rstd[:], rstd[:])
                inv = bigp.tile([D, 1], FP32, tag=f"inv{Li}", name=f"inv{Li}")
                nc.vector.tensor_tensor(out=inv[:], in0=rstd[:], in1=gamp[:],
                                        op=AOT.mult)
                nbias = bigp.tile([D, 1], FP32, tag=f"nb{Li}", name=f"nb{Li}")
                nc.vector.tensor_tensor(out=nbias[:], in0=mu, in1=inv[:],
                                        op=AOT.mult)
                nc.vector.tensor_tensor(out=nbias[:], in0=betp[:],
                                        in1=nbias[:], op=AOT.subtract)
                return inv, nbias

            gloc_sb = cpool.tile([P, len(L0["mm"])], BF)
            nc.scalar.dma_start(gloc_sb[:], gloc_t[0].ap()[:, :])
            iota3_sb = cpool.tile([P, WB, CB_OH], BF)
            nc.scalar.dma_start(
                iota3_sb[:].rearrange("p w c -> p (w c)"), iota3_t.ap()[:, :])

            def layer(Li, Ld):
                calls = Ld["calls"]
                mm = Ld["mm"]
                n_mm_b = Ld["n_mm_b"]

                table = x_pair_t.ap() if Li == 0 else x0p_full.ap()
                n_rows = Ld["n_rows"]

                stats_p = wpool.tile([D, NST, 6], FP32, tag="statsp")
                pool_ps = psC.tile([P, 2 * D], FP32, tag=f"pool{Li}",
                                   name=f"pool{Li}", bufs=1)
                pool_ps_l[Li] = pool_ps
                gci = [0]
                call_tile = {}
                mm_by_g = {}
                for e in mm:
                    mm_by_g.setdefault(e[0], []).append(e)
                seen_b = np.zeros(NB2, dtype=np.int64)

                ngrun = NG if max_groups is None else min(NG, max_groups)
                for g in range(ngrun):
                    blo, bhi = (g * 2 * GROUP_BLOCKS,
                                min((g + 1) * 2 * GROUP_BLOCKS, NB2))
                    # gathers first consumed by this group
                    for cid, (r, lo, hi, ico, gf) in enumerate(calls):
                        if gf != g:
                            continue
                        S = hi - lo
                        it = wpool.tile([P, idx_cols_max], I16, tag="idx",
                                        bufs=12)
                        nc.sync.dma_start(
                            it[:, :S // 16],
                            idx_t[Li].ap()[:, ico:ico + S // 16])
                        gt = gpool.tile([P, CALL_CHUNKS, D], BF, tag="gb")
                        base = r * RANGE
                        nrows_r = min(RANGE, n_rows - base)
                        # single-bf16 rows: gather 128B elems from the
                        # 256B-pitch table
                        nc.gpsimd.dma_gather(
                            gt[:, :S // P, :],
                            table[base:base + nrows_r, 0:D],
                            it[:, :S // 16],
                            S, S, D, elem_step=2 * D,
                        )
                        call_tile[cid] = gt

                    # scatter matmuls for this group
                    chl = mm_by_g.get(g, [])
                    ci0 = gci[0]
                    sts = sorted(set(b // (2 * ST_BLOCKS)
                                     for b in range(blo, bhi)))
                    stp = {st: psA.tile([P, ST_BLOCKS * P], FP32, tag="agg",
                                        name=f"agg{st}")
                           for st in sts}

                    oh_tiles = []
                    ng_ch = len(chl)
                    for cb0 in range(0, ng_ch, CB_OH):
                        n = min(CB_OH, ng_ch - cb0)
                        oh = ohpool.tile([P, WB, CB_OH], BF, tag="oh")
                        # [P, WB, CB] layout: all operands keep a packed
                        # (stride-1, 2-byte) last dim, unlocking the DVE
                        # 2x mode the old broadcast-last layout missed
                        nc.vector.tensor_tensor(
                            out=oh[:, :, :n],
                            in0=iota3_sb[:, :, :n],
                            in1=gloc_sb[:, ci0 + cb0:ci0 + cb0 + n]
                                .rearrange("p (x c) -> p x c", x=1)
                                .to_broadcast([P, WB, n]),
                            op=AOT.is_equal,
                        )
                        oh_tiles.append(oh)

                    for ci, (_, b, cid, col, _) in enumerate(chl):
                        gt = call_tile[cid]
                        oh = oh_tiles[ci // CB_OH]
                        st = b // (2 * ST_BLOCKS)
                        win = (b % (2 * ST_BLOCKS)) * WB
                        first = seen_b[b] == 0
                        last = seen_b[b] == n_mm_b[b] - 1
                        seen_b[b] += 1
                        nc.tensor.matmul(
                            stp[st][0:D, win:win + WB],
                            lhsT=gt[:, col, :],
                            rhs=oh[:, :, ci % CB_OH],
                            start=first, stop=last,
                        )
                    gci[0] += ng_ch

                    if skip_post:
                        continue
                    # supertile post-processing: copy, MLP, h, stats, pool
                    for st in sts:
                        sb0 = st * ST_BLOCKS
                        nwin = min(ST_BLOCKS, NB - sb0) * P
                        c0, c1 = sb0 * P, sb0 * P + nwin
                        agg_sb = wpool.tile([P, ST_BLOCKS * P], BF,
                                            tag="aggsb", bufs=2)
                        nc.scalar.copy(agg_sb[0:D, :nwin],
                                       stp[st][0:D, :nwin])
                        h1p = psB.tile([D, ST_BLOCKS * P], FP32, tag="mlp")
                        # bf16 matmuls: 1 cycle/row (vs 4 for fp32)
                        if Li == 0:
                            nc.tensor.matmul(h1p[:, :nwin],
                                             lhsT=w1s_sb[0][0:D, :],
                                             rhs=agg_sb[0:D, :nwin],
                                             start=True, stop=False)
                            xsl = wpool.tile([D, ST_BLOCKS * P], BF,
                                             tag="xsl", bufs=2)
                            nc.sync.dma_start(xsl[:, :nwin],
                                              xT_own_t.ap()[:, c0:c1])
                            nc.tensor.matmul(h1p[:, :nwin],
                                             lhsT=w1s_sb[0][0:D, :],
                                             rhs=xsl[:, :nwin],
                                             start=False, stop=True)
                        else:
                            nc.tensor.matmul(h1p[:, :nwin],
                                             lhsT=w1sc[0:D, :],
                                             rhs=agg_sb[0:D, :nwin],
                                             start=True, stop=False)
                            nc.tensor.matmul(h1p[:, :nwin],
                                             lhsT=w1sc[0:D, :],
                                             rhs=hT0[:, c0:c1],
                                             start=False, stop=False)
                            dsl = wpool.tile([1, ST_BLOCKS * P], BF,
                                             tag="dsl", bufs=2)
                            nc.sync.dma_start(dsl[:, :nwin],
                                              degp_t.ap()[st:st + 1, :nwin])
                            nc.tensor.matmul(h1p[:, :nwin],
                                             lhsT=vecd[:],
                                             rhs=dsl[:, :nwin],
                                             start=False, stop=True)
                        t1 = wpool.tile([D, ST_BLOCKS * P], BF, tag="t1",
                                        bufs=2)
                        nc.scalar.activation(t1[:, :nwin], h1p[:, :nwin],
                                             ACT.Tanh, bias=b1_sb[Li][:],
                                             scale=1.0)
                        h2p = psB.tile([D, ST_BLOCKS * P], FP32, tag="mlp")
                        nc.tensor.matmul(h2p[:, :nwin],
                                         lhsT=w2_sb[Li][:],
                                         rhs=t1[:, :nwin],
                                         start=True, stop=True)
                        if Li == 0:
                            ht_t = None
                            hts = hT0[:, c0:c1]
                        else:
                            ht_t = wpool.tile([D, ST_BLOCKS * P], BF,
                                              tag="ht1", bufs=2)
                            hts = ht_t[:, :nwin]
                        nc.scalar.activation(hts, h2p[:, :nwin],
                                             ACT.Tanh, bias=b2_sb[Li][:],
                                             scale=1.0)
                        # stats partials (exclude padded tail nodes)
                        r1 = min(c1, N_LOC)
                        if c0 < N_LOC:
                            hstat = (hT0[:, c0:r1] if Li == 0
                                     else ht_t[:, :r1 - c0])
                            nc.vector.bn_stats(out=stats_p[:, st, :],
                                               in_=hstat)
                        # bf16 transpose to node-major (PSUM); tanh already
                        # wrote bf16, so no staging copy is needed
                        tp = psC.tile([P, ST_BLOCKS, D], BF, tag="tp", bufs=1)
                        nbl = nwin // P
                        for j in range(nbl):
                            src = (hT0[:, c0 + j * P:c0 + (j + 1) * P]
                                   if Li == 0 else ht_t[:, j * P:(j + 1) * P])
                            nc.tensor.transpose(tp[:, j, :], src, ident[:])
                        xp = wpool.tile([P, ST_BLOCKS, D], BF,
                                        tag="xp", bufs=2)
                        nc.scalar.copy(xp[:, :nbl, :], tp[:, :nbl, :])
                        if Li == 0:
                            # single-bf16 writeback, clipped to N_LOC
                            nfull = max(0, min(c1, N_LOC) - c0) // P
                            if nfull:
                                nc.sync.dma_start(
                                    x0p_own.ap()[c0:c0 + nfull * P, 0:D]
                                    .rearrange("(j p) f -> p j f", p=P),
                                    xp[:, :nfull, :])
                            rem = min(c1, N_LOC) - (c0 + nfull * P)
                            if rem > 0:
                                nc.sync.dma_start(
                                    x0p_own.ap()[c0 + nfull * P:
                                                 c0 + nfull * P + rem, 0:D],
                                    xp[0:rem, nfull, :])
                        # pooling: one-hot matmul per block,
                        # accumulated in PSUM across the whole layer
                        for j in range(nbl):
                            b = sb0 + j
                            nc.tensor.matmul(
                                pool_ps[:, 0:D], lhsT=poh_all[:, b, :],
                                rhs=xp[:, j, :],
                                start=(b == 0), stop=(b == NB - 1))

                if skip_post:
                    return
                # ---- BN stats reduce + AllReduce ----
                mv = wpool.tile([D, 2], FP32, tag="mv")
                nc.vector.bn_aggr(out=mv[:], in_=stats_p[:])
                bpack = wpool.tile([D, 2], FP32, tag="bpack")
                nc.scalar.mul(bpack[:, 0:1], mv[:, 0:1], float(N_LOC))
                msq = wpool.tile([D, 1], FP32, tag="msq")
                nc.vector.tensor_tensor(out=msq[:], in0=mv[:, 0:1],
                                        in1=mv[:, 0:1], op=AOT.mult)
                nc.vector.tensor_tensor(out=msq[:], in0=mv[:, 1:2],
                                        in1=msq[:], op=AOT.add)
                nc.scalar.mul(bpack[:, 1:2], msq[:], float(N_LOC))
                nc.sync.dma_start(bn_stat[Li].ap()[:, :], bpack[:])
                if Li == 0:
                    nc.sync.dma_start(bn_in[0].ap()[:, :], bpack[:])
                if Li == 0 and not skip_cc:
                    # only layer 0's BN stats are needed on device (folded
                    # into L1's weights); L1's BN affine runs on the host
                    nc.gpsimd.collective_compute(
                        "AllReduce", AOT.add,
                        replica_groups=[list(range(N_CORES))],
                        ins=[bn_in[Li].ap().opt()],
                        outs=[bn_out[Li].ap().opt()],
                    )
                if Li == 0 and not skip_cc:
                    nc.gpsimd.collective_compute(
                        "AllGather", AOT.bypass,
                        replica_groups=[list(range(N_CORES))],
                        ins=[x0p_own.ap().opt()],
                        outs=[x0p_full.ap().opt()],
                    )
                # raw pool sums out (host applies the BN affine)
                nc.scalar.copy(osb[:, Li * D:(Li + 1) * D], pool_ps[:, 0:D])
                if Li == 0 and max_layers > 1:
                    inv, nbias = bn_coeffs(0)
                    # scale L1's stacked W1 by inv0; degree-bias row vector
                    nc.vector.tensor_scalar(
                        out=w1sc[0:D, :], in0=w1s_sb[1][0:D, :],
                        scalar1=inv[:], scalar2=None,
                        op0=AOT.mult)
                    nbias_bf = wpool.tile([D, 1], BF, tag="nbbf")
                    nc.vector.tensor_copy(nbias_bf[:], nbias[:])
                    vp = psC.tile([P, P], FP32, tag="misc", bufs=1)
                    nc.tensor.matmul(vp[0:1, 0:D], lhsT=nbias_bf[:],
                                     rhs=w1s_sb[1][0:D, :],
                                     start=True, stop=True)
                    nc.scalar.copy(vecd[:], vp[0:1, 0:D])

            layer(0, L0)
            if max_layers > 1:
                layer(1, L1)

            if not skip_post:
                nc.sync.dma_start(out_t.ap()[:, :], osb[:])

    nc.compile()
    return nc


def kernel(**inputs):
    from concourse.bass_utils import run_bass_kernel_spmd

    edge_index = np.asarray(inputs["edge_index"])
    batch = np.asarray(inputs["batch"])
    key = hashlib.sha1(
        edge_index.tobytes() + batch.tobytes()).hexdigest()
    if key not in _cache:
        struct = _prep_structure(edge_index, batch)
        nc = _build_program(struct)
        _cache[key] = (struct, nc)
    struct, nc = _cache[key]

    x = np.asarray(inputs["x"], dtype=np.float32)
    inv_perm = struct["inv_perm"]
    xp_all = x[inv_perm]                       # permuted node order
    x_pair = np.zeros((N_NODES, 2 * D), dtype=BF16)
    x_pair[:, 0:D] = xp_all.astype(BF16)
    in_maps = []
    for k in range(N_CORES):
        xT_own = np.zeros((D, N_PAD), dtype=BF16)
        xT_own[:, :N_LOC] = xp_all[k * N_LOC:(k + 1) * N_LOC].T.astype(BF16)
        m = dict(
            x_pair=x_pair,
            xT_own=xT_own,
            iotab=np.ascontiguousarray(
                np.tile(np.arange(P, dtype=np.float32), (P, CB))
                .reshape(P, CB * P).astype(BF16)),
            iota3=np.ascontiguousarray(np.tile(
                np.repeat(np.arange(WB, dtype=np.float32), CB_OH),
                (P, 1)).astype(BF16)),
            ploc=np.ascontiguousarray(struct["ploc"][k]),
            degp=np.ascontiguousarray(struct["degp"][k].astype(BF16)),
        )
        Ld = struct["layers"][0]
        m["idx_l0"] = np.ascontiguousarray(Ld["idx16"][k])
        m["gloc_l0"] = np.ascontiguousarray(Ld["gloc"][k])
        for i in range(2):
            W1 = np.asarray(inputs[f"W1_{i}"], dtype=np.float32)
            m[f"w1s_{i}"] = np.concatenate([W1, W1], axis=0).astype(BF16)
            m[f"w2_{i}"] = np.asarray(inputs[f"W2_{i}"], dtype=np.float32).astype(BF16)
            m[f"b1_{i}"] = np.asarray(inputs[f"b1_{i}"], dtype=np.float32).reshape(D, 1)
            m[f"b2_{i}"] = np.asarray(inputs[f"b2_{i}"], dtype=np.float32).reshape(D, 1)
            m[f"gamma_{i}"] = np.asarray(inputs[f"gamma_{i}"], dtype=np.float32).reshape(D, 1)
            m[f"beta_{i}"] = np.asarray(inputs[f"beta_{i}"], dtype=np.float32).reshape(D, 1)
        in_maps.append(m)

    res = run_bass_kernel_spmd(nc, in_maps, core_ids=list(range(N_CORES)))
    kernel.last_results = res

    # unshard: overlap-add raw per-core pool sums, then apply the BN
    # affine (device ships raw sums + per-core (sum, sumsq) stats)
    praw = np.zeros((NUM_GRAPHS, 2 * D), dtype=np.float64)
    for k in range(N_CORES):
        gb = struct["graph_base"][k]
        n = min(P, NUM_GRAPHS - gb)
        praw[gb:gb + n] += res.results[k]["pool"][:n]
    gcnt = np.bincount(batch.astype(np.int64),
                       minlength=NUM_GRAPHS).astype(np.float64)
    out = np.zeros((NUM_GRAPHS, 2 * D), dtype=np.float32)
    for i in range(2):
        S = np.zeros((D, 2), dtype=np.float64)
        for k in range(N_CORES):
            S += res.results[k][f"bnstat_{i}"]
        mu = S[:, 0] / N_NODES
        var = S[:, 1] / N_NODES - mu * mu
        gamma = np.asarray(inputs[f"gamma_{i}"], dtype=np.float64)
        beta = np.asarray(inputs[f"beta_{i}"], dtype=np.float64)
        inv = gamma / np.sqrt(var + BN_EPS)
        nbias = beta - mu * inv
        out[:, i * D:(i + 1) * D] = (
            praw[:, i * D:(i + 1) * D] * inv[None, :]
            + gcnt[:, None] * nbias[None, :]).astype(np.float32)
    return out

